# revision 16
# baseline (speedup 1.0000x reference)
"""Trainium2 Bass kernel for the HMM-style forward-algorithm problem.

Full-input contract: kernel(**inputs) takes the complete unsharded numpy
inputs and returns the full scalar output. Internally shards the batch N
over 8 NeuronCores (data parallel), with the vocab-LSE phase sharded over
V and combined with a tiny AllReduce.

Math (per core, NL=16 sequences):
  P = softmax(W_trans.reshape(K,K), axis=-1)            # transition probs
  LSE[j] = log sum_v exp(emb[j] @ W_cv[v])              # emission normalizer
  E[j, tn] = emb[j] @ W_cv[w[n,t]]                      # gathered logits
  ep_t = exp(E + SHIFT - LSE)                           # scaled emission probs
  s_0[i,n] = exp(W_start[i] + b_start[i])
  per step t=1..T-1:
     y_t = colsum(s_{t-1}); store y_t
     s_t = (P^T @ s_{t-1}) * ep_t * (1/y_t)
  yfin = colsum(s_{T-1}); store
  host: L[n] = sum_{t=2..T-1} log y_t + log yfin - (T-1)*SHIFT
  answer = -mean_n L[n]
"""
import os
from contextlib import ExitStack

import numpy as np

import concourse.bass as bass
import concourse.tile as tile
from concourse import bacc, mybir
from concourse import bass_utils
from concourse.masks import make_identity

N, T, K, V = 128, 512, 256, 50257
NCORES = 8
NL = N // NCORES          # 16 sequences per core
VPC = 6400                # padded vocab slice per core (50 chunks of 128)
NV_CHUNKS = VPC // 128    # 50
NG_CHUNKS = (T * NL) // 128  # 64 gather chunks
SHIFT = 50.0

FP = mybir.dt.float32
BF = mybir.dt.bfloat16
I32 = mybir.dt.int32

AF = mybir.ActivationFunctionType
ALU = mybir.AluOpType


def build_program(n_cores=NCORES, t_steps=T, v_total=V, vpc=VPC, nl=NL):
    """Builds the Bass program. Returns nc."""
    nv_chunks = vpc // 128
    ng_chunks = (t_steps * nl) // 128
    nc = bacc.Bacc(
        "TRN2", target_bir_lowering=False, debug=False, num_devices=n_cores
    )

    # ---- DRAM I/O ----
    wcv_full = nc.dram_tensor("wcv_full", [v_total, K], FP, kind="ExternalInput")
    wcv_slice = nc.dram_tensor("wcv_slice", [vpc, K], FP, kind="ExternalInput")
    idx_d = nc.dram_tensor("idx", [128, ng_chunks], I32, kind="ExternalInput")
    wtrans_d = nc.dram_tensor("wtrans", [K, K], FP, kind="ExternalInput")
    emb_d = nc.dram_tensor("emb", [K, K], FP, kind="ExternalInput")
    # row 0 = W_start[:,0], row 1 = b_start; summed inside the init matmul
    srows_d = nc.dram_tensor("srows", [2, K], FP, kind="ExternalInput")
    ybuf_d = nc.dram_tensor("ybuf", [1, t_steps * nl], FP, kind="ExternalOutput")

    with ExitStack() as ctx:
        tc = ctx.enter_context(tile.TileContext(nc))
        const = ctx.enter_context(tc.tile_pool(name="const", bufs=1))
        # scan SBUF pool opened early: the initial state tiles must share the
        # scan's s0/s1 slot families and outlive the setup/pre pools
        scan = ctx.enter_context(tc.tile_pool(name="scan", bufs=3))

        # ---------- constants ----------
        ident_f = const.tile([128, 128], FP)
        make_identity(nc, ident_f[:])
        ident = const.tile([128, 128], BF)
        nc.vector.tensor_copy(ident[:], ident_f[:])
        ones128 = const.tile([128, 128], BF)
        nc.gpsimd.memset(ones128[:], 1.0)
        onesn = const.tile([2, nl], BF)
        nc.gpsimd.memset(onesn[:], 1.0)

        ybuf = const.tile([1, t_steps * nl], FP)

        # big persistent buffers: emission logits (bf16), [j, t*nl+n]
        E0 = const.tile([128, t_steps * nl], BF)
        E1 = const.tile([128, t_steps * nl], BF)
        Ebuf = [E0, E1]

        with tc.tile_pool(name="setup", bufs=2) as setup, tc.tile_pool(
            name="setup_ps", bufs=2, space="PSUM"
        ) as setup_ps:
            # ---------- embT: 4 bf16 tiles embT[ch][jh] = emb[jh-blk, ch-blk]^T ----
            embT = [[None, None], [None, None]]
            for jh in range(2):
                embrow = setup.tile([128, K], FP, tag="embrow")
                nc.sync.dma_start(embrow[:], emb_d.ap()[jh * 128:(jh + 1) * 128, :])
                embrow_b = setup.tile([128, K], BF, tag="embrowb")
                nc.vector.tensor_copy(embrow_b[:], embrow[:])
                for ch in range(2):
                    tp = setup_ps.tile([128, 128], BF, tag="tp")
                    nc.tensor.transpose(
                        tp[:], embrow_b[:, ch * 128:(ch + 1) * 128], ident[:]
                    )
                    dst = const.tile([128, 128], BF, name=f"embT_{ch}_{jh}")
                    nc.vector.tensor_copy(dst[:], tp[:])
                    embT[ch][jh] = dst

            # ---------- P tiles: P_i[ih] (128, 256) bf16, rows i, cols j ----------
            P_i = []
            for ih in range(2):
                wt_f = setup.tile([128, K], FP, tag="wt")
                nc.sync.dma_start(wt_f[:], wtrans_d.ap()[ih * 128:(ih + 1) * 128, :])
                wt_e = setup.tile([128, K], FP, tag="wte")
                nc.scalar.activation(wt_e[:], wt_f[:], AF.Exp)
                rs = setup.tile([128, 1], FP, tag="rs")
                nc.vector.reduce_sum(out=rs[:], in_=wt_e[:], axis=mybir.AxisListType.X)
                rrs = setup.tile([128, 1], FP, tag="rrs")
                nc.vector.reciprocal(rrs[:], rs[:])
                pt = const.tile([128, K], BF, name=f"P_i{ih}")
                nc.scalar.mul(pt[:], wt_e[:], rrs[:])
                P_i.append(pt)

            # ---------- initial state s0 (2 x (128, nl) bf16) ----------
            # s0[i,n] = exp(ws[i] + bs[i]): the matmul contracts over the two
            # srows rows (computing ws+bs) while broadcasting across n.
            srows_f = setup.tile([2, K], FP, tag="srowsf")
            nc.sync.dma_start(srows_f[:], srows_d.ap()[:, :])
            srows_b = setup.tile([2, K], BF, tag="srowsb")
            nc.vector.tensor_copy(srows_b[:], srows_f[:])

            s_cur = []
            for ih in range(2):
                sp = setup_ps.tile([128, nl], FP, tag="s0ps")
                nc.tensor.matmul(
                    sp[:],
                    lhsT=srows_b[0:2, ih * 128:(ih + 1) * 128],
                    rhs=onesn[:],
                    start=True,
                    stop=True,
                )
                st = scan.tile([128, nl], BF, name=f"sinit{ih}", tag=f"s{ih}", bufs=3)
                nc.scalar.activation(st[:], sp[:], AF.Exp)
                s_cur.append(st)

            # ---------- idx table ----------
            idxt = const.tile([128, ng_chunks], I32)
            nc.sync.dma_start(idxt[:], idx_d.ap()[:, :])

        # ---------- phase 1 + phase 2 (shared psum/work pools) ----------
        accw = [const.tile([128, nv_chunks], FP, name=f"accw{h}") for h in range(2)]
        bias_t = [const.tile([128, 1], FP, name=f"bias{h}") for h in range(2)]

        with tc.tile_pool(name="pre", bufs=3) as pre, tc.tile_pool(
            name="pre_ps", bufs=3, space="PSUM"
        ) as pre_ps:
            # ---- phase 2 emission logits: gather chunks, transpose, matmul ----
            for g in range(ng_chunks):
                gg = pre.tile([128, K], FP, tag="gg")
                nc.gpsimd.indirect_dma_start(
                    out=gg[:],
                    out_offset=None,
                    in_=wcv_full.ap(),
                    in_offset=bass.IndirectOffsetOnAxis(
                        ap=idxt[:, g:g + 1], axis=0
                    ),
                )
                ggb = pre.tile([128, K], BF, tag="ggb")
                nc.vector.tensor_copy(ggb[:], gg[:])
                gT = []
                for ch in range(2):
                    tp2 = pre_ps.tile([128, 128], BF, tag="tp2")
                    nc.tensor.transpose(
                        tp2[:], ggb[:, ch * 128:(ch + 1) * 128], ident[:]
                    )
                    gt = pre.tile([128, 128], BF, tag=f"gT{ch}")
                    nc.vector.tensor_copy(gt[:], tp2[:])
                    gT.append(gt)
                for jh in range(2):
                    ep_ps = pre_ps.tile([128, 128], FP, tag="mmout")
                    nc.tensor.matmul(
                        ep_ps[:], lhsT=embT[0][jh][:], rhs=gT[0][:],
                        start=True, stop=False,
                    )
                    nc.tensor.matmul(
                        ep_ps[:], lhsT=embT[1][jh][:], rhs=gT[1][:],
                        start=False, stop=True,
                    )
                    nc.scalar.copy(
                        Ebuf[jh][:, g * 128:(g + 1) * 128], ep_ps[:]
                    )

            # ---- phase 1 LSE over the vocab slice ----
            for g in range(nv_chunks):
                wslc = pre.tile([128, K], FP, tag="wslc")
                nc.sync.dma_start(
                    wslc[:], wcv_slice.ap()[g * 128:(g + 1) * 128, :]
                )
                wslb = pre.tile([128, K], BF, tag="wslb")
                nc.vector.tensor_copy(wslb[:], wslc[:])
                wT = []
                for ch in range(2):
                    tp3 = pre_ps.tile([128, 128], BF, tag="tp2")
                    nc.tensor.transpose(
                        tp3[:], wslb[:, ch * 128:(ch + 1) * 128], ident[:]
                    )
                    wt_ = pre.tile([128, 128], BF, tag=f"wT{ch}")
                    nc.vector.tensor_copy(wt_[:], tp3[:])
                    wT.append(wt_)
                for jh in range(2):
                    l_ps = pre_ps.tile([128, 128], FP, tag="mmout")
                    nc.tensor.matmul(
                        l_ps[:], lhsT=embT[0][jh][:], rhs=wT[0][:],
                        start=True, stop=False,
                    )
                    nc.tensor.matmul(
                        l_ps[:], lhsT=embT[1][jh][:], rhs=wT[1][:],
                        start=False, stop=True,
                    )
                    junk = pre.tile([128, 128], FP, tag="junk")
                    nc.scalar.activation(
                        junk[:], l_ps[:], AF.Exp,
                        accum_out=accw[jh][:, g:g + 1],
                    )

            # ---- totals + allreduce + bias ----
            totals = const.tile([128, 2], FP)
            for jh in range(2):
                nc.vector.reduce_sum(
                    out=totals[:, jh:jh + 1], in_=accw[jh][:],
                    axis=mybir.AxisListType.X,
                )
            if n_cores > 1:
                with tc.tile_pool(name="dram", bufs=1, space="DRAM") as dram:
                    cin = dram.tile([128, 2], FP)
                    cout = dram.tile([128, 2], FP)
                    nc.gpsimd.dma_start(cin[:], totals[:])
                    nc.gpsimd.collective_compute(
                        "AllReduce",
                        ALU.add,
                        replica_groups=[list(range(n_cores))],
                        ins=[cin.opt()],
                        outs=[cout.opt()],
                    )
                    allred = const.tile([128, 2], FP)
                    nc.gpsimd.dma_start(allred[:], cout[:])
            else:
                allred = totals
            # ScalarE Ln only accepts |x| <= 2^64; totals reach ~1e23, so
            # compute Ln(x * 2^-32) and fold the +32*ln2 into the bias.
            LN2_32 = 32.0 * float(np.log(2.0))
            for jh in range(2):
                lse = pre.tile([128, 1], FP, tag="lse")
                nc.scalar.activation(
                    lse[:], allred[:, jh:jh + 1], AF.Ln, scale=2.0 ** -32
                )
                # bias = SHIFT - (lse + 32*ln2)
                nc.vector.tensor_scalar(
                    out=bias_t[jh][:], in0=lse[:],
                    scalar1=-1.0, scalar2=SHIFT - LN2_32,
                    op0=ALU.mult, op1=ALU.add,
                )

        # ---------- phase 3: the scan ----------
        with tc.tile_pool(name="scan_ps", bufs=2, space="PSUM") as scan_ps:

            def emit_y(step_slot, s_pair):
                yb = scan_ps.tile([128, nl], FP, tag="yb")
                nc.tensor.matmul(
                    yb[:], lhsT=ones128[:], rhs=s_pair[0][:], start=True, stop=False
                )
                nc.tensor.matmul(
                    yb[:], lhsT=ones128[:], rhs=s_pair[1][:], start=False, stop=True
                )
                nc.scalar.copy(
                    ybuf[0:1, step_slot * nl:(step_slot + 1) * nl], yb[0:1, :]
                )
                return yb

            for t in range(1, t_steps):
                yb = emit_y(t - 1, s_cur)
                rt = scan.tile([128, nl], FP, tag="rt")
                nc.vector.reciprocal(rt[:], yb[:])
                # ep tiles for this step
                eps = []
                for jh in range(2):
                    ept = scan.tile([128, nl], BF, tag=f"ep{jh}")
                    nc.scalar.activation(
                        ept[:], Ebuf[jh][:, t * nl:(t + 1) * nl], AF.Exp,
                        bias=bias_t[jh][:],
                    )
                    eps.append(ept)
                s_next = []
                for jh in range(2):
                    cur = scan_ps.tile([128, nl], FP, tag=f"cur{jh}")
                    nc.tensor.matmul(
                        cur[:],
                        lhsT=P_i[0][:, jh * 128:(jh + 1) * 128],
                        rhs=s_cur[0][:],
                        start=True, stop=False,
                    )
                    nc.tensor.matmul(
                        cur[:],
                        lhsT=P_i[1][:, jh * 128:(jh + 1) * 128],
                        rhs=s_cur[1][:],
                        start=False, stop=True,
                    )
                    mt = scan.tile([128, nl], FP, tag=f"mt{jh}")
                    nc.vector.tensor_mul(mt[:], cur[:], rt[:])
                    sn = scan.tile([128, nl], BF, tag=f"s{jh}", bufs=3)
                    nc.vector.tensor_mul(sn[:], mt[:], eps[jh][:])
                    s_next.append(sn)
                s_cur = s_next

            # final colsum
            emit_y(t_steps - 1, s_cur)

        # output
        nc.sync.dma_start(ybuf_d.ap()[:, :], ybuf[:])

    nc.compile()
    return nc


# ------------------------------------------------------------------
# host-side wrapper
# ------------------------------------------------------------------
_cache = {}


def _get_program():
    if "nc" not in _cache:
        _cache["nc"] = build_program()
    return _cache["nc"]


def _make_in_maps(w, emb_cluster_W, W_cv, W_start, b_start, W_trans):
    w = np.asarray(w)
    wcv = np.ascontiguousarray(np.asarray(W_cv, np.float32))
    wcv_pad = np.zeros((VPC * NCORES, K), np.float32)
    wcv_pad[:V] = wcv
    emb = np.ascontiguousarray(np.asarray(emb_cluster_W, np.float32))
    wtrans = np.ascontiguousarray(
        np.asarray(W_trans, np.float32).reshape(K, K)
    )
    srows = np.ascontiguousarray(
        np.stack([np.asarray(W_start, np.float32)[:, 0],
                  np.asarray(b_start, np.float32)])
    )

    rr = np.arange(T * NL)
    tt = rr // NL
    nn = rr % NL

    in_maps = []
    for c in range(NCORES):
        wsl = w[c * NL:(c + 1) * NL, :]  # (NL, T)
        flat = np.asarray(wsl[nn, tt], np.int32)  # (T*NL,) t-major
        idx = np.ascontiguousarray(flat.reshape(NG_CHUNKS, 128).T)  # (128, NG)
        in_maps.append(
            {
                "wcv_full": wcv,
                "wcv_slice": np.ascontiguousarray(
                    wcv_pad[c * VPC:(c + 1) * VPC]
                ),
                "idx": idx,
                "wtrans": wtrans,
                "emb": emb,
                "srows": srows,
            }
        )
    return in_maps


def _finish(results):
    """results: list of per-core output dicts with 'ybuf' (1, T*NL)."""
    Ls = []
    for c in range(NCORES):
        y = np.asarray(results[c]["ybuf"], np.float64).reshape(T, NL)
        # slot 0 = y_1 (excluded: cancels Z0); slots 1..T-2 = y_2..y_{T-1};
        # slot T-1 = final colsum
        L = np.log(y[1:]).sum(axis=0) - (T - 1) * SHIFT
        Ls.append(L)
    Lall = np.concatenate(Ls)
    return np.float32(-np.mean(Lall))


def kernel(w, emb_cluster_W, W_cv, W_start, b_start, W_trans):
    nc = _get_program()
    in_maps = _make_in_maps(w, emb_cluster_W, W_cv, W_start, b_start, W_trans)
    res = bass_utils.run_bass_kernel_spmd(
        nc, in_maps, core_ids=list(range(NCORES))
    )
    return _finish(res.results)


if __name__ == "__main__":
    data = np.load(os.path.join(os.path.dirname(__file__), "inputs.npz"))
    out = kernel(**{k: data[k] for k in data.files})
    print("kernel output:", out)


# revision 23
# speedup vs baseline: 146.5325x; 146.5325x over previous
"""Trainium2 Bass kernel for the HMM-style forward-algorithm problem.

Full-input contract: kernel(**inputs) takes the complete unsharded numpy
inputs and returns the full scalar output. Internally shards the batch N
over 8 NeuronCores (data parallel), with the vocab-LSE phase sharded over
V and combined with a tiny AllReduce.

Math (per core, NL=16 sequences):
  P = softmax(W_trans.reshape(K,K), axis=-1)            # transition probs
  LSE[j] = log sum_v exp(emb[j] @ W_cv[v])              # emission normalizer
  E[j, tn] = emb[j] @ W_cv[w[n,t]]                      # gathered logits
  ep_t = exp(E + SHIFT - LSE)                           # scaled emission probs
  s_0[i,n] = exp(W_start[i] + b_start[i])
  per step t=1..T-1:
     y_t = colsum(s_{t-1}); store y_t
     s_t = (P^T @ s_{t-1}) * ep_t * (1/y_t)
  yfin = colsum(s_{T-1}); store
  host: L[n] = sum_{t=2..T-1} log y_t + log yfin - (T-1)*SHIFT
  answer = -mean_n L[n]
"""
import os
from contextlib import ExitStack

import numpy as np

import concourse.bass as bass
import concourse.tile as tile
from concourse import bacc, mybir
from concourse import bass_utils
from concourse.masks import make_identity

N, T, K, V = 128, 512, 256, 50257
NCORES = 8
NL = N // NCORES          # 16 sequences per core
VPC = 6400                # padded vocab slice per core (50 chunks of 128)
NV_CHUNKS = VPC // 128    # 50
NG_CHUNKS = (T * NL) // 128  # 64 gather chunks
SHIFT = 50.0

FP = mybir.dt.float32
BF = mybir.dt.bfloat16
I32 = mybir.dt.int32

AF = mybir.ActivationFunctionType
ALU = mybir.AluOpType


def n_norm_steps(t_steps, norm_every):
    return len(range(1, t_steps, norm_every))


def build_program(n_cores=NCORES, t_steps=T, v_total=V, vpc=VPC, nl=NL,
                  norm_every=2):
    """Builds the Bass program. Returns nc."""
    nv_chunks = vpc // 128
    ng_chunks = (t_steps * nl) // 128
    n_norm = n_norm_steps(t_steps, norm_every)
    nc = bacc.Bacc(
        "TRN2", target_bir_lowering=False, debug=False, num_devices=n_cores
    )

    # ---- DRAM I/O ----
    wcv_full = nc.dram_tensor("wcv_full", [v_total, K], FP, kind="ExternalInput")
    wcv_slice = nc.dram_tensor("wcv_slice", [vpc, K], FP, kind="ExternalInput")
    idx_d = nc.dram_tensor("idx", [128, ng_chunks], I32, kind="ExternalInput")
    wtrans_d = nc.dram_tensor("wtrans", [K, K], FP, kind="ExternalInput")
    emb_d = nc.dram_tensor("emb", [K, K], FP, kind="ExternalInput")
    # row 0 = W_start[:,0], row 1 = b_start; summed inside the init matmul
    srows_d = nc.dram_tensor("srows", [2, K], FP, kind="ExternalInput")
    ybuf_d = nc.dram_tensor(
        "ybuf", [1, (n_norm + 1) * nl], FP, kind="ExternalOutput"
    )

    with ExitStack() as ctx:
        tc = ctx.enter_context(tile.TileContext(nc))
        const = ctx.enter_context(tc.tile_pool(name="const", bufs=1))
        # scan SBUF pool opened early: the initial state tiles must share the
        # scan's s0/s1 slot families and outlive the setup/pre pools
        scan = ctx.enter_context(tc.tile_pool(name="scan", bufs=3))

        # ---------- constants ----------
        ident_f = const.tile([128, 128], FP)
        make_identity(nc, ident_f[:])
        ident = const.tile([128, 128], BF)
        nc.vector.tensor_copy(ident[:], ident_f[:])
        ones128 = const.tile([128, 128], BF)
        nc.gpsimd.memset(ones128[:], 1.0)
        onesn = const.tile([2, nl], BF)
        nc.gpsimd.memset(onesn[:], 1.0)

        ybuf = const.tile([1, (n_norm + 1) * nl], FP)

        # big persistent buffers: emission logits (bf16), [j, t*nl+n]
        E0 = const.tile([128, t_steps * nl], BF)
        E1 = const.tile([128, t_steps * nl], BF)
        Ebuf = [E0, E1]
        # scaled emission probs exp(E + SHIFT - LSE)
        EPT0 = const.tile([128, t_steps * nl], BF)
        EPT1 = const.tile([128, t_steps * nl], BF)
        EPT = [EPT0, EPT1]

        with tc.tile_pool(name="setup", bufs=2) as setup, tc.tile_pool(
            name="setup_ps", bufs=2, space="PSUM"
        ) as setup_ps:
            # ---------- embT: 4 bf16 tiles embT[ch][jh] = emb[jh-blk, ch-blk]^T ----
            embT = [[None, None], [None, None]]
            for jh in range(2):
                embrow = setup.tile([128, K], FP, tag="embrow")
                nc.sync.dma_start(embrow[:], emb_d.ap()[jh * 128:(jh + 1) * 128, :])
                embrow_b = setup.tile([128, K], BF, tag="embrowb")
                nc.vector.tensor_copy(embrow_b[:], embrow[:])
                for ch in range(2):
                    tp = setup_ps.tile([128, 128], BF, tag="tp")
                    nc.tensor.transpose(
                        tp[:], embrow_b[:, ch * 128:(ch + 1) * 128], ident[:]
                    )
                    dst = const.tile([128, 128], BF, name=f"embT_{ch}_{jh}")
                    nc.vector.tensor_copy(dst[:], tp[:])
                    embT[ch][jh] = dst

            # ---------- P tiles: P_i[ih] (128, 256) bf16, rows i, cols j ----------
            P_i = []
            for ih in range(2):
                wt_f = setup.tile([128, K], FP, tag="wt")
                nc.sync.dma_start(wt_f[:], wtrans_d.ap()[ih * 128:(ih + 1) * 128, :])
                wt_e = setup.tile([128, K], FP, tag="wte")
                nc.scalar.activation(wt_e[:], wt_f[:], AF.Exp)
                rs = setup.tile([128, 1], FP, tag="rs")
                nc.vector.reduce_sum(out=rs[:], in_=wt_e[:], axis=mybir.AxisListType.X)
                rrs = setup.tile([128, 1], FP, tag="rrs")
                nc.vector.reciprocal(rrs[:], rs[:])
                pt = const.tile([128, K], BF, name=f"P_i{ih}")
                nc.scalar.mul(pt[:], wt_e[:], rrs[:])
                P_i.append(pt)

            # ---------- initial state s0 (2 x (128, nl) bf16) ----------
            # s0[i,n] = exp(ws[i] + bs[i]): the matmul contracts over the two
            # srows rows (computing ws+bs) while broadcasting across n.
            srows_f = setup.tile([2, K], FP, tag="srowsf")
            nc.sync.dma_start(srows_f[:], srows_d.ap()[:, :])
            srows_b = setup.tile([2, K], BF, tag="srowsb")
            nc.vector.tensor_copy(srows_b[:], srows_f[:])

            s_cur = []
            for ih in range(2):
                sp = setup_ps.tile([128, nl], FP, tag="s0ps")
                nc.tensor.matmul(
                    sp[:],
                    lhsT=srows_b[0:2, ih * 128:(ih + 1) * 128],
                    rhs=onesn[:],
                    start=True,
                    stop=True,
                )
                st = scan.tile([128, nl], BF, name=f"sinit{ih}", tag=f"s{ih}", bufs=3)
                nc.scalar.activation(st[:], sp[:], AF.Exp)
                s_cur.append(st)

            # ---------- idx table ----------
            idxt = const.tile([128, ng_chunks], I32)
            nc.sync.dma_start(idxt[:], idx_d.ap()[:, :])

        # ---------- phase 1 + phase 2 (shared psum/work pools) ----------
        accw = [const.tile([128, nv_chunks], FP, name=f"accw{h}") for h in range(2)]
        bias_t = [const.tile([128, 1], FP, name=f"bias{h}") for h in range(2)]

        with tc.tile_pool(name="pre", bufs=3) as pre, tc.tile_pool(
            name="pre_ps", bufs=3, space="PSUM"
        ) as pre_ps:
            # ---- phase 2 emission logits: gather chunks, transpose, matmul ----
            for g in range(ng_chunks):
                gg = pre.tile([128, K], FP, tag="gg")
                nc.gpsimd.indirect_dma_start(
                    out=gg[:],
                    out_offset=None,
                    in_=wcv_full.ap(),
                    in_offset=bass.IndirectOffsetOnAxis(
                        ap=idxt[:, g:g + 1], axis=0
                    ),
                )
                ggb = pre.tile([128, K], BF, tag="ggb")
                nc.vector.tensor_copy(ggb[:], gg[:])
                gT = []
                for ch in range(2):
                    tp2 = pre_ps.tile([128, 128], BF, tag="tp2")
                    nc.tensor.transpose(
                        tp2[:], ggb[:, ch * 128:(ch + 1) * 128], ident[:]
                    )
                    gt = pre.tile([128, 128], BF, tag=f"gT{ch}")
                    nc.vector.tensor_copy(gt[:], tp2[:])
                    gT.append(gt)
                for jh in range(2):
                    ep_ps = pre_ps.tile([128, 128], FP, tag="mmout")
                    nc.tensor.matmul(
                        ep_ps[:], lhsT=embT[0][jh][:], rhs=gT[0][:],
                        start=True, stop=False,
                    )
                    nc.tensor.matmul(
                        ep_ps[:], lhsT=embT[1][jh][:], rhs=gT[1][:],
                        start=False, stop=True,
                    )
                    nc.scalar.copy(
                        Ebuf[jh][:, g * 128:(g + 1) * 128], ep_ps[:]
                    )

            # ---- phase 1 LSE over the vocab slice ----
            for g in range(nv_chunks):
                wslc = pre.tile([128, K], FP, tag="wslc")
                nc.sync.dma_start(
                    wslc[:], wcv_slice.ap()[g * 128:(g + 1) * 128, :]
                )
                wslb = pre.tile([128, K], BF, tag="wslb")
                nc.vector.tensor_copy(wslb[:], wslc[:])
                wT = []
                for ch in range(2):
                    tp3 = pre_ps.tile([128, 128], BF, tag="tp2")
                    nc.tensor.transpose(
                        tp3[:], wslb[:, ch * 128:(ch + 1) * 128], ident[:]
                    )
                    wt_ = pre.tile([128, 128], BF, tag=f"wT{ch}")
                    nc.vector.tensor_copy(wt_[:], tp3[:])
                    wT.append(wt_)
                for jh in range(2):
                    l_ps = pre_ps.tile([128, 128], FP, tag="mmout")
                    nc.tensor.matmul(
                        l_ps[:], lhsT=embT[0][jh][:], rhs=wT[0][:],
                        start=True, stop=False,
                    )
                    nc.tensor.matmul(
                        l_ps[:], lhsT=embT[1][jh][:], rhs=wT[1][:],
                        start=False, stop=True,
                    )
                    junk = pre.tile([128, 128], FP, tag="junk")
                    nc.scalar.activation(
                        junk[:], l_ps[:], AF.Exp,
                        accum_out=accw[jh][:, g:g + 1],
                    )

            # ---- totals + allreduce + bias ----
            totals = const.tile([128, 2], FP)
            for jh in range(2):
                nc.vector.reduce_sum(
                    out=totals[:, jh:jh + 1], in_=accw[jh][:],
                    axis=mybir.AxisListType.X,
                )
            if n_cores > 1:
                with tc.tile_pool(name="dram", bufs=1, space="DRAM") as dram:
                    cin = dram.tile([128, 2], FP)
                    cout = dram.tile([128, 2], FP)
                    nc.gpsimd.dma_start(cin[:], totals[:])
                    nc.gpsimd.collective_compute(
                        "AllReduce",
                        ALU.add,
                        replica_groups=[list(range(n_cores))],
                        ins=[cin.opt()],
                        outs=[cout.opt()],
                    )
                    allred = const.tile([128, 2], FP)
                    nc.gpsimd.dma_start(allred[:], cout[:])
            else:
                allred = totals
            # ScalarE Ln only accepts |x| <= 2^64; totals reach ~1e23, so
            # compute Ln(x * 2^-32) and fold the +32*ln2 into the bias.
            LN2_32 = 32.0 * float(np.log(2.0))
            for jh in range(2):
                lse = pre.tile([128, 1], FP, tag="lse")
                nc.scalar.activation(
                    lse[:], allred[:, jh:jh + 1], AF.Ln, scale=2.0 ** -32
                )
                # bias = SHIFT - (lse + 32*ln2)
                nc.vector.tensor_scalar(
                    out=bias_t[jh][:], in0=lse[:],
                    scalar1=-1.0, scalar2=SHIFT - LN2_32,
                    op0=ALU.mult, op1=ALU.add,
                )

            # materialize EPT = exp(E + bias), chunked so the scan can start
            # after the first chunk
            ECH = min(2048, t_steps * nl)
            for jh in range(2):
                for c0 in range(0, t_steps * nl, ECH):
                    nc.scalar.activation(
                        EPT[jh][:, c0:c0 + ECH], Ebuf[jh][:, c0:c0 + ECH],
                        AF.Exp, bias=bias_t[jh][:],
                    )

        # ---------- phase 3: the scan ----------
        with tc.tile_pool(name="scan_ps", bufs=2, space="PSUM") as scan_ps:

            def emit_y(step_slot, s_pair):
                yb = scan_ps.tile([128, nl], FP, tag="yb")
                nc.tensor.matmul(
                    yb[:], lhsT=ones128[:], rhs=s_pair[0][:], start=True, stop=False
                )
                nc.tensor.matmul(
                    yb[:], lhsT=ones128[:], rhs=s_pair[1][:], start=False, stop=True
                )
                nc.scalar.copy(
                    ybuf[0:1, step_slot * nl:(step_slot + 1) * nl], yb[0:1, :]
                )
                return yb

            for t in range(1, t_steps):
                is_norm = (t - 1) % norm_every == 0
                if is_norm:
                    yb = emit_y((t - 1) // norm_every, s_cur)
                    rt = scan.tile([128, nl], FP, tag="rt")
                    nc.vector.reciprocal(rt[:], yb[:])
                s_next = []
                for jh in range(2):
                    cur = scan_ps.tile([128, nl], FP, tag=f"cur{jh}")
                    nc.tensor.matmul(
                        cur[:],
                        lhsT=P_i[0][:, jh * 128:(jh + 1) * 128],
                        rhs=s_cur[0][:],
                        start=True, stop=False,
                    )
                    nc.tensor.matmul(
                        cur[:],
                        lhsT=P_i[1][:, jh * 128:(jh + 1) * 128],
                        rhs=s_cur[1][:],
                        start=False, stop=True,
                    )
                    ep_slice = EPT[jh][:, t * nl:(t + 1) * nl]
                    sn = scan.tile([128, nl], BF, tag=f"s{jh}", bufs=3)
                    if is_norm:
                        mt = scan.tile([128, nl], FP, tag=f"mt{jh}")
                        nc.vector.tensor_mul(mt[:], cur[:], rt[:])
                        nc.vector.tensor_mul(sn[:], mt[:], ep_slice)
                    else:
                        nc.vector.tensor_mul(sn[:], cur[:], ep_slice)
                    s_next.append(sn)
                s_cur = s_next

            # final colsum
            emit_y(n_norm, s_cur)

        # output
        nc.sync.dma_start(ybuf_d.ap()[:, :], ybuf[:])

    nc.compile()
    return nc


# ------------------------------------------------------------------
# host-side wrapper
# ------------------------------------------------------------------
_cache = {}


def _get_program():
    if "nc" not in _cache:
        _cache["nc"] = build_program()
    return _cache["nc"]


def _make_in_maps(w, emb_cluster_W, W_cv, W_start, b_start, W_trans):
    w = np.asarray(w)
    wcv = np.ascontiguousarray(np.asarray(W_cv, np.float32))
    wcv_pad = np.zeros((VPC * NCORES, K), np.float32)
    wcv_pad[:V] = wcv
    emb = np.ascontiguousarray(np.asarray(emb_cluster_W, np.float32))
    wtrans = np.ascontiguousarray(
        np.asarray(W_trans, np.float32).reshape(K, K)
    )
    srows = np.ascontiguousarray(
        np.stack([np.asarray(W_start, np.float32)[:, 0],
                  np.asarray(b_start, np.float32)])
    )

    rr = np.arange(T * NL)
    tt = rr // NL
    nn = rr % NL

    in_maps = []
    for c in range(NCORES):
        wsl = w[c * NL:(c + 1) * NL, :]  # (NL, T)
        flat = np.asarray(wsl[nn, tt], np.int32)  # (T*NL,) t-major
        idx = np.ascontiguousarray(flat.reshape(NG_CHUNKS, 128).T)  # (128, NG)
        in_maps.append(
            {
                "wcv_full": wcv,
                "wcv_slice": np.ascontiguousarray(
                    wcv_pad[c * VPC:(c + 1) * VPC]
                ),
                "idx": idx,
                "wtrans": wtrans,
                "emb": emb,
                "srows": srows,
            }
        )
    return in_maps


def _finish(results, t_steps=T, norm_every=2):
    """results: list of per-core output dicts with 'ybuf'."""
    n_norm = n_norm_steps(t_steps, norm_every)
    Ls = []
    for c in range(NCORES):
        y = np.asarray(results[c]["ybuf"], np.float64).reshape(n_norm + 1, NL)
        # slot 0 = y_1 = Z0 (excluded; cancels the unnormalized s_0 scale);
        # slots 1..n_norm-1 = applied normalizers; slot n_norm = final colsum
        L = np.log(y[1:]).sum(axis=0) - (t_steps - 1) * SHIFT
        Ls.append(L)
    Lall = np.concatenate(Ls)
    return np.float32(-np.mean(Lall))


def kernel(w, emb_cluster_W, W_cv, W_start, b_start, W_trans):
    nc = _get_program()
    in_maps = _make_in_maps(w, emb_cluster_W, W_cv, W_start, b_start, W_trans)
    res = bass_utils.run_bass_kernel_spmd(
        nc, in_maps, core_ids=list(range(NCORES))
    )
    return _finish(res.results)


if __name__ == "__main__":
    data = np.load(os.path.join(os.path.dirname(__file__), "inputs.npz"))
    out = kernel(**{k: data[k] for k in data.files})
    print("kernel output:", out)
